# revision 31
# baseline (speedup 1.0000x reference)
"""BerryPhaseInterferometer Trainium2 kernel.

Math notes (derived from the reference):
  - knot = base*(h+delta) - base*(h-delta) = 2*delta*base  (h cancels), so
    knot = tanh(W' @ [x;1]) with W' = 2*delta ⊙ [W | b] folded on host.
  - Re(inner) = sum_d v_state[d] * t_state[d]  over all 256 dims
    Im(inner) = sum_d v_state[d] * (Q @ t_state)[d], Q the pairwise
    (2i,2i+1) swap-with-sign matrix.
  - det(S/eps + I) = 1 + inner/eps  (rank-1 update), so
    angle(det) = atan2(Im, Re + eps)  (positive scaling keeps the angle).
  - berry = atan2(Im, Re);  eta = mean(angle)/pi.

Layout on device: activations stored (dim=128 partitions, rows free),
one (128, 512) tile per half/path; v and t paths share F/G weights.
GELU(exact erf) is built from the Erf activation; a leading dummy Erf
pins the ACT table set to sigmoid_and_others which holds ALL needed
functions (tanh, erf, arctan) -> exactly one table load.
"""

import math

import numpy as np

import concourse.bacc as bacc
import concourse.bass as bass
import concourse.mybir as mybir
from concourse.bass_utils import run_bass_kernel_spmd
from concourse.tile import TileContext

F32 = mybir.dt.float32
AF = mybir.ActivationFunctionType
ALU = mybir.AluOpType

B = 4096
DIM = 256
H = 128
NCORES = 8
RPC = B // NCORES  # rows per core = 512
EPS = 1e-6
INV_SQRT2 = 1.0 / math.sqrt(2.0)
PI = math.pi

# wblob column layout (128 partitions)
_C_FT = 0
_C_GT = 128
_C_QT = 256
_C_B = 384  # Fbe, Gbe, Fbr, Gbr (one column each)
WBLOB_W = 388
# xblob (compact): rows 0:4 = visT | vW ; rows 4:6 = txtT | tW
# (on SBUF the txt rows land at partitions 32:34 for PE row-group packing)
_C_X = 0
_C_W = RPC  # 512
XBLOB_W = RPC + DIM  # 768
XBLOB_P = 6

_cached = {}
last_results = None  # BassKernelResults of the most recent run (for test.py)


def _build_nc(zero_bias):
    nc = bacc.Bacc()

    wblob = nc.declare_dram_parameter("wblob", [H, WBLOB_W], F32, isOutput=False)
    xblob = nc.declare_dram_parameter("xblob", [XBLOB_P, XBLOB_W], F32, isOutput=False)
    berry = nc.declare_dram_parameter("berry2d", [H, 4], F32, isOutput=True)
    eta_part = nc.declare_dram_parameter("eta_part", [1], F32, isOutput=True)

    with TileContext(nc) as tc:
        with (
            tc.tile_pool(name="consts", bufs=1) as consts,
            tc.tile_pool(name="work", bufs=1) as work,
            tc.tile_pool(name="px", bufs=3, space="PSUM") as px,
            tc.tile_pool(name="pf", bufs=3, space="PSUM") as pf,
            tc.tile_pool(name="pt", bufs=1, space="PSUM") as ptp,
        ):
            xb = consts.tile([34, XBLOB_W], F32, tag="xb")
            wb = consts.tile([H, WBLOB_W], F32, tag="wb")
            # trigger on different engines so the queues start pumping at once
            nc.sync.dma_start(out=xb[0:4, :], in_=xblob[0:4, :])
            nc.sync.dma_start(out=xb[32:34, :], in_=xblob[4:6, :])
            nc.scalar.dma_start(out=wb[0:64, :], in_=wblob[0:64, :])
            nc.gpsimd.dma_start(out=wb[64:128, :], in_=wblob[64:128, :])

            visT = xb[0:4, _C_X : _C_X + RPC]
            txtT = xb[32:34, _C_X : _C_X + RPC]
            vW = xb[0:4, _C_W : _C_W + DIM]
            tW = xb[32:34, _C_W : _C_W + DIM]
            FT = wb[:, _C_FT : _C_FT + H]
            GT = wb[:, _C_GT : _C_GT + H]
            QT = wb[:, _C_QT : _C_QT + H]
            Fbe = wb[:, _C_B + 0 : _C_B + 1]
            Gbe = wb[:, _C_B + 1 : _C_B + 2]
            Fbr = wb[:, _C_B + 2 : _C_B + 3]
            Gbr = wb[:, _C_B + 3 : _C_B + 4]

            ones_s = consts.tile([H, 1], F32, tag="ones")
            nc.vector.memset(ones_s, 1.0)
            nones_s = consts.tile([H, 1], F32, tag="nones")
            nc.vector.memset(nones_s, -1.0)

            # dummy erf first: pins ACT table set to sigmoid_and_others
            scratch = work.tile([1, 1], F32, tag="scratch")
            nc.scalar.activation(out=scratch, in_=ones_s[0:1, :], func=AF.Erf)

            # PE warmup: wide matmuls on row-groups 64-127 flip the HAM clock
            # gate to 8/8. One accumulation group (no WAW drains between them)
            # and disjoint row groups from the expand matmuls (rows 0-3/32-33),
            # so the real expands stream concurrently as soon as data lands.
            jx = consts.tile([H, RPC], F32, tag="jx")
            nc.vector.memset(jx, 1.0)
            pwu = ptp.tile([H, RPC], F32, tag="ptB")
            NWU = 3
            wu_mms = [
                lambda i=i: nc.tensor.matmul(
                    pwu, jx[64:H, 0:H], jx[64:H, :],
                    start=(i == 0), stop=(i == NWU - 1),
                )
                for i in range(NWU)
            ]

            erf_kw = dict(scale=INV_SQRT2)
            if not zero_bias:
                erf_kw_F = dict(scale=INV_SQRT2, bias=Fbe)
                erf_kw_G = dict(scale=INV_SQRT2, bias=Gbe)
            else:
                erf_kw_F = erf_kw_G = erf_kw

            def gelu_combine(y, e, p, k, braw):
                """y = k + gelu(p + b)   with e = erf((p+b)/sqrt2) already done.

                zero_bias: gF = (e+1)*p ; y = 0.5*gF + k     (2 DVE ops)
                else:      e1 = e+1 ; g = (p+b)*e1 ; y = 0.5*g + k
                """
                g = work.tile([H, RPC], F32, tag="g")
                if zero_bias:
                    nc.vector.scalar_tensor_tensor(
                        out=g, in0=e, scalar=1.0, in1=p, op0=ALU.add, op1=ALU.mult
                    )
                else:
                    nc.vector.tensor_scalar_add(out=e, in0=e, scalar1=1.0)
                    nc.vector.scalar_tensor_tensor(
                        out=g, in0=p, scalar=braw, in1=e, op0=ALU.add, op1=ALU.mult
                    )
                nc.vector.scalar_tensor_tensor(
                    out=y, in0=g, scalar=0.5, in1=k, op0=ALU.mult, op1=ALU.add
                )

            # ---- expand: psum = W' @ [x;1]  (v at rows 0-3, t at rows 32-33:
            # different PE row groups -> the v/t matmuls run concurrently,
            # interleaved with the warmup streams on rows 64-127) ----
            pX2_v = px.tile([H, RPC], F32, tag="px")
            pX2_t = px.tile([H, RPC], F32, tag="px")
            pX1_v = px.tile([H, RPC], F32, tag="px")
            pX1_t = px.tile([H, RPC], F32, tag="px")
            wu_mms[0]()
            nc.tensor.matmul(pX2_v, vW[:, H:DIM], visT, start=True, stop=True)
            nc.tensor.matmul(pX2_t, tW[:, H:DIM], txtT, start=True, stop=True)
            wu_mms[1]()
            nc.tensor.matmul(pX1_v, vW[:, 0:H], visT, start=True, stop=True)
            nc.tensor.matmul(pX1_t, tW[:, 0:H], txtT, start=True, stop=True)
            wu_mms[2]()

            # ---- knots: k = tanh(psum) ----
            k2_v = work.tile([H, RPC], F32, tag="k2_v")
            k2_t = work.tile([H, RPC], F32, tag="k2_t")
            k1_v = work.tile([H, RPC], F32, tag="k1_v")
            k1_t = work.tile([H, RPC], F32, tag="k1_t")
            nc.scalar.activation(out=k2_v, in_=pX2_v, func=AF.Tanh)
            nc.scalar.activation(out=k2_t, in_=pX2_t, func=AF.Tanh)

            # ---- F stage: y1 = k1 + gelu(F @ k2 + Fb) ----
            pF_v = pf.tile([H, RPC], F32, tag="pf")
            nc.tensor.matmul(pF_v, FT, k2_v, start=True, stop=True)
            eF_v = work.tile([H, RPC], F32, tag="eF_v")
            nc.scalar.activation(out=eF_v, in_=pF_v, func=AF.Erf, **erf_kw_F)
            nc.scalar.activation(out=k1_v, in_=pX1_v, func=AF.Tanh)

            pF_t = pf.tile([H, RPC], F32, tag="pf")
            nc.tensor.matmul(pF_t, FT, k2_t, start=True, stop=True)
            y1_v = work.tile([H, RPC], F32, tag="y1_v")
            gelu_combine(y1_v, eF_v, pF_v, k1_v, Fbr)

            eF_t = work.tile([H, RPC], F32, tag="eF_t")
            nc.scalar.activation(out=k1_t, in_=pX1_t, func=AF.Tanh)
            nc.scalar.activation(out=eF_t, in_=pF_t, func=AF.Erf, **erf_kw_F)

            # ---- G stage: y2 = k2 + gelu(G @ y1 + Gb) ----
            pG_v = pf.tile([H, RPC], F32, tag="pf")
            nc.tensor.matmul(pG_v, GT, y1_v, start=True, stop=True)
            eG_v = work.tile([H, RPC], F32, tag="eG_v")
            nc.scalar.activation(out=eG_v, in_=pG_v, func=AF.Erf, **erf_kw_G)

            y1_t = work.tile([H, RPC], F32, tag="y1_t")
            gelu_combine(y1_t, eF_t, pF_t, k1_t, Fbr)

            y2_v = work.tile([H, RPC], F32, tag="y2_v")
            gelu_combine(y2_v, eG_v, pG_v, k2_v, Gbr)

            pG_t = pf.tile([H, RPC], F32, tag="pf")
            nc.tensor.matmul(pG_t, GT, y1_t, start=True, stop=True)
            eG_t = work.tile([H, RPC], F32, tag="eG_t")
            nc.scalar.activation(out=eG_t, in_=pG_t, func=AF.Erf, **erf_kw_G)
            y2_t = work.tile([H, RPC], F32, tag="y2_t")
            gelu_combine(y2_t, eG_t, pG_t, k2_t, Gbr)

            # ---- u = Q @ v_state (pairwise swap-with-sign; the v path is ready
            # ~2us before t, so this runs off the critical tail).
            # Im = sum v*(Q t) = -sum t*(Q v): the minus sign is folded into the
            # Im reduce matmuls via a -1 vector. Reuses px slots. ----
            u1 = px.tile([H, RPC], F32, tag="px")
            u2 = px.tile([H, RPC], F32, tag="px")
            nc.tensor.matmul(u1, QT, y1_v, start=True, stop=True)
            nc.tensor.matmul(u2, QT, y2_v, start=True, stop=True)

            # ---- products ----
            zre1 = work.tile([H, RPC], F32, tag="zre1")
            zre2 = work.tile([H, RPC], F32, tag="zre2")
            zim1 = work.tile([H, RPC], F32, tag="zim1")
            zim2 = work.tile([H, RPC], F32, tag="zim2")
            nc.vector.tensor_mul(zre1, y1_v, y1_t)
            nc.vector.tensor_mul(zim1, y1_t, u1)
            # half2 products chunked so each reduce matmul can fire asap
            for c in range(4):
                lo, hi = c * H, (c + 1) * H
                nc.vector.tensor_mul(zre2[:, lo:hi], y2_v[:, lo:hi], y2_t[:, lo:hi])
                nc.vector.tensor_mul(zim2[:, lo:hi], y2_t[:, lo:hi], u2[:, lo:hi])

            # ---- partition reduction, transposed: ptX[:, c] = sum_d z[d, 128c+p]
            # cols 0-3 Re, 4-7 Im; one single-shot matmul per column per half,
            # halves combined on DVE (avoids interleaved PSUM accum groups) ----
            ptA = ptp.tile([H, 16], F32, tag="ptA")
            ptB = ptp.tile([H, 8], F32, tag="ptB")
            for c in range(4):
                lo, hi = c * H, (c + 1) * H
                nc.tensor.matmul(ptA[:, c : c + 1], zre1[:, lo:hi], ones_s,
                                 start=True, stop=True)
                nc.tensor.matmul(ptA[:, 4 + c : 5 + c], zim1[:, lo:hi], nones_s,
                                 start=True, stop=True)
            for c in range(4):
                lo, hi = c * H, (c + 1) * H
                nc.tensor.matmul(ptB[:, c : c + 1], zre2[:, lo:hi], ones_s,
                                 start=True, stop=True)
                nc.tensor.matmul(ptB[:, 4 + c : 5 + c], zim2[:, lo:hi], nones_s,
                                 start=True, stop=True)
            sA = work.tile([H, 8], F32, tag="sA")
            nc.vector.tensor_copy(out=sA, in_=ptA[:, 0:8])
            S8 = work.tile([H, 8], F32, tag="S8")
            nc.vector.tensor_add(S8, sA, ptB)

            # ---- atan2 tail on (128, 8): cols 0-3 berry, 4-7 eta ----
            D = work.tile([H, 8], F32, tag="D")
            nc.vector.tensor_copy(out=D[:, 0:4], in_=S8[:, 0:4])
            nc.vector.tensor_scalar_add(out=D[:, 4:8], in0=S8[:, 0:4], scalar1=EPS)
            R8 = work.tile([H, 8], F32, tag="R8")
            nc.vector.reciprocal(out=R8, in_=D)
            q = work.tile([H, 8], F32, tag="q")
            nc.vector.tensor_mul(q[:, 0:4], S8[:, 4:8], R8[:, 0:4])
            nc.vector.tensor_mul(q[:, 4:8], S8[:, 4:8], R8[:, 4:8])
            a8 = work.tile([H, 8], F32, tag="a8")
            nc.scalar.activation(out=a8, in_=q, func=AF.Arctan)
            n8 = work.tile([H, 8], F32, tag="n8")
            nc.vector.tensor_scalar(
                out=n8, in0=D, scalar1=0.0, scalar2=None, op0=ALU.is_lt
            )
            s4 = work.tile([H, 4], F32, tag="s4")
            nc.vector.tensor_scalar(
                out=s4, in0=S8[:, 4:8], scalar1=0.0, scalar2=2.0, op0=ALU.is_gt, op1=ALU.mult
            )
            w8 = work.tile([H, 8], F32, tag="w8")
            nc.vector.scalar_tensor_tensor(
                out=w8[:, 0:4], in0=s4, scalar=-1.0, in1=n8[:, 0:4], op0=ALU.add, op1=ALU.mult
            )
            nc.vector.scalar_tensor_tensor(
                out=w8[:, 4:8], in0=s4, scalar=-1.0, in1=n8[:, 4:8], op0=ALU.add, op1=ALU.mult
            )
            ang = work.tile([H, 8], F32, tag="ang")
            nc.vector.scalar_tensor_tensor(
                out=ang, in0=w8, scalar=PI, in1=a8, op0=ALU.mult, op1=ALU.add
            )

            # ---- outputs ----
            # contiguous (128,4) layout; host transposes to row order
            nc.sync.dma_start(out=berry[:, :], in_=ang[:, 0:4])

            esum = work.tile([H, 1], F32, tag="esum")
            nc.vector.tensor_reduce(
                out=esum, in_=ang[:, 4:8], axis=mybir.AxisListType.X, op=ALU.add
            )
            nc.tensor.matmul(ptA[0:1, 8:9], esum, ones_s, start=True, stop=True)
            eta_sb = work.tile([1, 1], F32, tag="eta_sb")
            nc.vector.tensor_copy(out=eta_sb, in_=ptA[0:1, 8:9])
            nc.sync.dma_start(out=eta_part[:], in_=eta_sb)

    nc.compile()
    return nc


def _prep_in_maps(
    vision_ycbcr, text_bytes, v_proj_w, v_proj_b, v_h, v_delta,
    t_proj_w, t_proj_b, t_h, t_delta, F_w, F_b, G_w, G_b,
):
    f = np.float32
    vision = np.ascontiguousarray(vision_ycbcr, dtype=f)  # (B, 3)
    text = np.ascontiguousarray(text_bytes, dtype=f)  # (B, 1)

    # augmented transposed inputs: [x; 1]
    visT = np.concatenate([vision.T, np.ones((1, B), f)], axis=0)  # (4, B)
    txtT = np.concatenate([text.T, np.ones((1, B), f)], axis=0)  # (2, B)

    # knot = tanh(2*delta*(W x + b)) -> fold 2*delta into [W | b]
    dv = 2.0 * np.asarray(v_delta, dtype=f).reshape(DIM, 1)
    dt_ = 2.0 * np.asarray(t_delta, dtype=f).reshape(DIM, 1)
    vW_aug = dv * np.concatenate(
        [np.asarray(v_proj_w, f), np.asarray(v_proj_b, f).reshape(DIM, 1)], axis=1
    )  # (256, 4)
    tW_aug = dt_ * np.concatenate(
        [np.asarray(t_proj_w, f), np.asarray(t_proj_b, f).reshape(DIM, 1)], axis=1
    )  # (256, 2)

    FT = np.asarray(F_w, f).T  # (128,128) lhsT
    GT = np.asarray(G_w, f).T
    Fb = np.asarray(F_b, f).reshape(H)
    Gb = np.asarray(G_b, f).reshape(H)
    zero_bias = not (np.any(Fb) or np.any(Gb))

    # Q: u = Q @ t  with u[2i] = -t[2i+1], u[2i+1] = t[2i]; lhsT = Q.T
    Q = np.zeros((H, H), f)
    idx = np.arange(0, H, 2)
    Q[idx, idx + 1] = -1.0
    Q[idx + 1, idx] = 1.0

    wb = np.zeros((H, WBLOB_W), f)
    wb[:, _C_FT : _C_FT + H] = FT
    wb[:, _C_GT : _C_GT + H] = GT
    wb[:, _C_QT : _C_QT + H] = Q.T
    wb[:, _C_B + 0] = Fb * f(INV_SQRT2)
    wb[:, _C_B + 1] = Gb * f(INV_SQRT2)
    wb[:, _C_B + 2] = Fb
    wb[:, _C_B + 3] = Gb

    xbase = np.zeros((XBLOB_P, XBLOB_W), f)
    xbase[0:4, _C_W : _C_W + DIM] = vW_aug.T
    xbase[4:6, _C_W : _C_W + DIM] = tW_aug.T

    in_maps = []
    for c in range(NCORES):
        sl = slice(c * RPC, (c + 1) * RPC)
        xb = xbase.copy()
        xb[0:4, _C_X : _C_X + RPC] = visT[:, sl]
        xb[4:6, _C_X : _C_X + RPC] = txtT[:, sl]
        in_maps.append({"wblob": wb, "xblob": xb})
    return in_maps, zero_bias


def kernel(tmpdir=None, **inputs):
    global last_results
    in_maps, zero_bias = _prep_in_maps(**inputs)
    key = ("v2", zero_bias)
    if key not in _cached:
        _cached[key] = _build_nc(zero_bias)
    res = run_bass_kernel_spmd(_cached[key], in_maps, list(range(NCORES)), tmpdir=tmpdir)
    last_results = res
    # berry2d[p, c] holds row c*128+p of this core's 512-row block
    berry = np.concatenate(
        [np.ascontiguousarray(res.results[c]["berry2d"].T).reshape(RPC) for c in range(NCORES)]
    )
    eta_sum = np.sum([res.results[c]["eta_part"][0] for c in range(NCORES)], dtype=np.float64)
    eta = np.float32(eta_sum / (B * np.pi))
    return berry.astype(np.float32), eta


# revision 34
# speedup vs baseline: 1.1355x; 1.1355x over previous
"""BerryPhaseInterferometer Trainium2 kernel.

Math notes (derived from the reference):
  - knot = base*(h+delta) - base*(h-delta) = 2*delta*base  (h cancels), so
    knot = tanh(W' @ [x;1]) with W' = 2*delta ⊙ [W | b] folded on host.
  - Re(inner) = sum_d v_state[d] * t_state[d]  over all 256 dims
    Im(inner) = sum_d v_state[d] * (Q @ t_state)[d], Q the pairwise
    (2i,2i+1) swap-with-sign matrix.
  - det(S/eps + I) = 1 + inner/eps  (rank-1 update), so
    angle(det) = atan2(Im, Re + eps)  (positive scaling keeps the angle).
  - berry = atan2(Im, Re);  eta = mean(angle)/pi.

Layout on device: activations stored (dim=128 partitions, rows free),
one (128, 512) tile per half/path; v and t paths share F/G weights.
GELU(exact erf) is built from the Erf activation; a leading dummy Erf
pins the ACT table set to sigmoid_and_others which holds ALL needed
functions (tanh, erf, arctan) -> exactly one table load.
"""

import math

import numpy as np

import concourse.bacc as bacc
import concourse.bass as bass
import concourse.mybir as mybir
from concourse.bass_utils import run_bass_kernel_spmd
from concourse.tile import TileContext

F32 = mybir.dt.float32
AF = mybir.ActivationFunctionType
ALU = mybir.AluOpType

B = 4096
DIM = 256
H = 128
NCORES = 8
RPC = B // NCORES  # rows per core = 512
EPS = 1e-6
INV_SQRT2 = 1.0 / math.sqrt(2.0)
PI = math.pi

# wblob column layout (128 partitions)
_C_FT = 0
_C_GT = 128
_C_QT = 256
_C_B = 384  # Fbe, Gbe, Fbr, Gbr (one column each)
WBLOB_W = 388
# xblob (compact): rows 0:4 = visT | vW ; rows 4:6 = txtT | tW
# (on SBUF the txt rows land at partitions 32:34 for PE row-group packing)
_C_X = 0
_C_W = RPC  # 512
XBLOB_W = RPC + DIM  # 768
XBLOB_P = 6

_cached = {}
last_results = None  # BassKernelResults of the most recent run (for test.py)


def _build_nc(zero_bias):
    nc = bacc.Bacc()

    wblob = nc.declare_dram_parameter("wblob", [H, WBLOB_W], F32, isOutput=False)
    xblob = nc.declare_dram_parameter("xblob", [XBLOB_P, XBLOB_W], F32, isOutput=False)
    berry = nc.declare_dram_parameter("berry2d", [H, 4], F32, isOutput=True)
    eta_part = nc.declare_dram_parameter("eta_part", [1], F32, isOutput=True)

    with TileContext(nc) as tc:
        with (
            tc.tile_pool(name="consts", bufs=1) as consts,
            tc.tile_pool(name="work", bufs=1) as work,
            tc.tile_pool(name="px", bufs=3, space="PSUM") as px,
            tc.tile_pool(name="pf", bufs=3, space="PSUM") as pf,
            tc.tile_pool(name="pt", bufs=1, space="PSUM") as ptp,
        ):
            xb = consts.tile([34, XBLOB_W], F32, tag="xb")
            wb = consts.tile([H, WBLOB_W], F32, tag="wb")
            # trigger on different engines so the queues start pumping at once
            nc.sync.dma_start(out=xb[0:4, :], in_=xblob[0:4, :])
            nc.sync.dma_start(out=xb[32:34, :], in_=xblob[4:6, :])
            nc.scalar.dma_start(out=wb[0:64, :], in_=wblob[0:64, :])
            nc.gpsimd.dma_start(out=wb[64:128, :], in_=wblob[64:128, :])

            visT = xb[0:4, _C_X : _C_X + RPC]
            txtT = xb[32:34, _C_X : _C_X + RPC]
            vW = xb[0:4, _C_W : _C_W + DIM]
            tW = xb[32:34, _C_W : _C_W + DIM]
            FT = wb[:, _C_FT : _C_FT + H]
            GT = wb[:, _C_GT : _C_GT + H]
            QT = wb[:, _C_QT : _C_QT + H]
            Fbe = wb[:, _C_B + 0 : _C_B + 1]
            Gbe = wb[:, _C_B + 1 : _C_B + 2]
            Fbr = wb[:, _C_B + 2 : _C_B + 3]
            Gbr = wb[:, _C_B + 3 : _C_B + 4]

            ones_s = consts.tile([H, 1], F32, tag="ones")
            nc.vector.memset(ones_s, 1.0)
            nones_s = consts.tile([H, 1], F32, tag="nones")
            nc.vector.memset(nones_s, -1.0)

            # dummy erf first: pins ACT table set to sigmoid_and_others
            scratch = work.tile([1, 1], F32, tag="scratch")
            nc.scalar.activation(out=scratch, in_=ones_s[0:1, :], func=AF.Erf)

            # PE warmup: wide matmuls on row-groups 64-127 flip the HAM clock
            # gate to 8/8. One accumulation group (no WAW drains between them)
            # and disjoint row groups from the expand matmuls (rows 0-3/32-33),
            # so the real expands stream concurrently as soon as data lands.
            jx = consts.tile([H, RPC], F32, tag="jx")
            nc.vector.memset(jx, 1.0)
            pwu = ptp.tile([H, RPC], F32, tag="ptB")
            NWU = 2
            for i in range(NWU):
                nc.tensor.matmul(
                    pwu, jx[:, 0:H], jx, start=(i == 0), stop=(i == NWU - 1)
                )

            erf_kw = dict(scale=INV_SQRT2)
            if not zero_bias:
                erf_kw_F = dict(scale=INV_SQRT2, bias=Fbe)
                erf_kw_G = dict(scale=INV_SQRT2, bias=Gbe)
            else:
                erf_kw_F = erf_kw_G = erf_kw

            def gelu_combine(y, e, p, k, braw):
                """y = k + gelu(p + b)   with e = erf((p+b)/sqrt2) already done.

                zero_bias: gF = (e+1)*p ; y = 0.5*gF + k     (2 DVE ops)
                else:      e1 = e+1 ; g = (p+b)*e1 ; y = 0.5*g + k
                """
                g = work.tile([H, RPC], F32, tag="g")
                if zero_bias:
                    nc.vector.scalar_tensor_tensor(
                        out=g, in0=e, scalar=1.0, in1=p, op0=ALU.add, op1=ALU.mult
                    )
                else:
                    nc.vector.tensor_scalar_add(out=e, in0=e, scalar1=1.0)
                    nc.vector.scalar_tensor_tensor(
                        out=g, in0=p, scalar=braw, in1=e, op0=ALU.add, op1=ALU.mult
                    )
                nc.vector.scalar_tensor_tensor(
                    out=y, in0=g, scalar=0.5, in1=k, op0=ALU.mult, op1=ALU.add
                )

            # ---- expand: psum = W' @ [x;1]  (v at rows 0-3, t at rows 32-33:
            # different PE row groups -> the v/t matmuls run concurrently,
            # interleaved with the warmup streams on rows 64-127) ----
            pX2_v = px.tile([H, RPC], F32, tag="px")
            pX2_t = px.tile([H, RPC], F32, tag="px")
            pX1_v = px.tile([H, RPC], F32, tag="px")
            pX1_t = px.tile([H, RPC], F32, tag="px")
            nc.tensor.matmul(pX2_v, vW[:, H:DIM], visT, start=True, stop=True)
            nc.tensor.matmul(pX2_t, tW[:, H:DIM], txtT, start=True, stop=True)
            nc.tensor.matmul(pX1_v, vW[:, 0:H], visT, start=True, stop=True)
            nc.tensor.matmul(pX1_t, tW[:, 0:H], txtT, start=True, stop=True)

            # ---- knots: k = tanh(psum) ----
            k2_v = work.tile([H, RPC], F32, tag="k2_v")
            k2_t = work.tile([H, RPC], F32, tag="k2_t")
            k1_v = work.tile([H, RPC], F32, tag="k1_v")
            k1_t = work.tile([H, RPC], F32, tag="k1_t")
            nc.scalar.activation(out=k2_v, in_=pX2_v, func=AF.Tanh)
            nc.scalar.activation(out=k2_t, in_=pX2_t, func=AF.Tanh)

            # ---- F stage: y1 = k1 + gelu(F @ k2 + Fb) ----
            pF_v = pf.tile([H, RPC], F32, tag="pf")
            nc.tensor.matmul(pF_v, FT, k2_v, start=True, stop=True)
            eF_v = work.tile([H, RPC], F32, tag="eF_v")
            nc.scalar.activation(out=eF_v, in_=pF_v, func=AF.Erf, **erf_kw_F)
            nc.scalar.activation(out=k1_v, in_=pX1_v, func=AF.Tanh)

            pF_t = pf.tile([H, RPC], F32, tag="pf")
            nc.tensor.matmul(pF_t, FT, k2_t, start=True, stop=True)
            y1_v = work.tile([H, RPC], F32, tag="y1_v")
            gelu_combine(y1_v, eF_v, pF_v, k1_v, Fbr)

            eF_t = work.tile([H, RPC], F32, tag="eF_t")
            nc.scalar.activation(out=k1_t, in_=pX1_t, func=AF.Tanh)
            nc.scalar.activation(out=eF_t, in_=pF_t, func=AF.Erf, **erf_kw_F)

            # ---- G stage: y2 = k2 + gelu(G @ y1 + Gb) ----
            pG_v = pf.tile([H, RPC], F32, tag="pf")
            nc.tensor.matmul(pG_v, GT, y1_v, start=True, stop=True)
            # u = Q @ v_state (pairwise swap-with-sign); Im = -sum t*(Q v),
            # minus sign folded into the Im reduce matmuls via nones_s
            u1 = px.tile([H, RPC], F32, tag="px")
            nc.tensor.matmul(u1, QT, y1_v, start=True, stop=True)
            eG_v = work.tile([H, RPC], F32, tag="eG_v")
            nc.scalar.activation(out=eG_v, in_=pG_v, func=AF.Erf, **erf_kw_G)

            y1_t = work.tile([H, RPC], F32, tag="y1_t")
            gelu_combine(y1_t, eF_t, pF_t, k1_t, Fbr)

            y2_v = work.tile([H, RPC], F32, tag="y2_v")
            gelu_combine(y2_v, eG_v, pG_v, k2_v, Gbr)

            pG_t = pf.tile([H, RPC], F32, tag="pf")
            nc.tensor.matmul(pG_t, GT, y1_t, start=True, stop=True)
            u2 = px.tile([H, RPC], F32, tag="px")
            nc.tensor.matmul(u2, QT, y2_v, start=True, stop=True)
            eG_t = work.tile([H, RPC], F32, tag="eG_t")
            nc.scalar.activation(out=eG_t, in_=pG_t, func=AF.Erf, **erf_kw_G)
            y2_t = work.tile([H, RPC], F32, tag="y2_t")
            gelu_combine(y2_t, eG_t, pG_t, k2_t, Gbr)

            # ---- products ----
            zre1 = work.tile([H, RPC], F32, tag="zre1")
            zre2 = work.tile([H, RPC], F32, tag="zre2")
            zim1 = work.tile([H, RPC], F32, tag="zim1")
            zim2 = work.tile([H, RPC], F32, tag="zim2")
            nc.vector.tensor_mul(zre1, y1_v, y1_t)
            nc.vector.tensor_mul(zim1, y1_t, u1)
            # half2 products chunked so each reduce matmul can fire asap
            for c in range(4):
                lo, hi = c * H, (c + 1) * H
                nc.vector.tensor_mul(zre2[:, lo:hi], y2_v[:, lo:hi], y2_t[:, lo:hi])
                nc.vector.tensor_mul(zim2[:, lo:hi], y2_t[:, lo:hi], u2[:, lo:hi])

            # ---- partition reduction, transposed: ptX[:, c] = sum_d z[d, 128c+p]
            # cols 0-3 Re, 4-7 Im; one single-shot matmul per column per half,
            # halves combined on DVE (avoids interleaved PSUM accum groups) ----
            ptA = ptp.tile([H, 16], F32, tag="ptA")
            ptB = ptp.tile([H, 8], F32, tag="ptB")
            for c in range(4):
                lo, hi = c * H, (c + 1) * H
                nc.tensor.matmul(ptA[:, c : c + 1], zre1[:, lo:hi], ones_s,
                                 start=True, stop=True)
                nc.tensor.matmul(ptA[:, 4 + c : 5 + c], zim1[:, lo:hi], nones_s,
                                 start=True, stop=True)
            for c in range(4):
                lo, hi = c * H, (c + 1) * H
                nc.tensor.matmul(ptB[:, c : c + 1], zre2[:, lo:hi], ones_s,
                                 start=True, stop=True)
                nc.tensor.matmul(ptB[:, 4 + c : 5 + c], zim2[:, lo:hi], nones_s,
                                 start=True, stop=True)
            sA = work.tile([H, 8], F32, tag="sA")
            nc.vector.tensor_copy(out=sA, in_=ptA[:, 0:8])
            S8 = work.tile([H, 8], F32, tag="S8")
            nc.vector.tensor_add(S8, sA, ptB)

            # ---- atan2 tail on (128, 8): cols 0-3 berry, 4-7 eta ----
            D = work.tile([H, 8], F32, tag="D")
            nc.vector.tensor_copy(out=D[:, 0:4], in_=S8[:, 0:4])
            nc.vector.tensor_scalar_add(out=D[:, 4:8], in0=S8[:, 0:4], scalar1=EPS)
            R8 = work.tile([H, 8], F32, tag="R8")
            nc.vector.reciprocal(out=R8, in_=D)
            q = work.tile([H, 8], F32, tag="q")
            nc.vector.tensor_mul(q[:, 0:4], S8[:, 4:8], R8[:, 0:4])
            nc.vector.tensor_mul(q[:, 4:8], S8[:, 4:8], R8[:, 4:8])
            a8 = work.tile([H, 8], F32, tag="a8")
            nc.scalar.activation(out=a8, in_=q, func=AF.Arctan)
            n8 = work.tile([H, 8], F32, tag="n8")
            nc.vector.tensor_scalar(
                out=n8, in0=D, scalar1=0.0, scalar2=None, op0=ALU.is_lt
            )
            s4 = work.tile([H, 4], F32, tag="s4")
            nc.vector.tensor_scalar(
                out=s4, in0=S8[:, 4:8], scalar1=0.0, scalar2=2.0, op0=ALU.is_gt, op1=ALU.mult
            )
            w8 = work.tile([H, 8], F32, tag="w8")
            nc.vector.scalar_tensor_tensor(
                out=w8[:, 0:4], in0=s4, scalar=-1.0, in1=n8[:, 0:4], op0=ALU.add, op1=ALU.mult
            )
            nc.vector.scalar_tensor_tensor(
                out=w8[:, 4:8], in0=s4, scalar=-1.0, in1=n8[:, 4:8], op0=ALU.add, op1=ALU.mult
            )
            ang = work.tile([H, 8], F32, tag="ang")
            nc.vector.scalar_tensor_tensor(
                out=ang, in0=w8, scalar=PI, in1=a8, op0=ALU.mult, op1=ALU.add
            )

            # ---- outputs ----
            # contiguous (128,4) layout; host transposes to row order
            nc.sync.dma_start(out=berry[:, :], in_=ang[:, 0:4])

            esum = work.tile([H, 1], F32, tag="esum")
            nc.vector.tensor_reduce(
                out=esum, in_=ang[:, 4:8], axis=mybir.AxisListType.X, op=ALU.add
            )
            nc.tensor.matmul(ptA[0:1, 8:9], esum, ones_s, start=True, stop=True)
            eta_sb = work.tile([1, 1], F32, tag="eta_sb")
            nc.vector.tensor_copy(out=eta_sb, in_=ptA[0:1, 8:9])
            nc.sync.dma_start(out=eta_part[:], in_=eta_sb)

    nc.compile()
    return nc


def _prep_in_maps(
    vision_ycbcr, text_bytes, v_proj_w, v_proj_b, v_h, v_delta,
    t_proj_w, t_proj_b, t_h, t_delta, F_w, F_b, G_w, G_b,
):
    f = np.float32
    vision = np.ascontiguousarray(vision_ycbcr, dtype=f)  # (B, 3)
    text = np.ascontiguousarray(text_bytes, dtype=f)  # (B, 1)

    # augmented transposed inputs: [x; 1]
    visT = np.concatenate([vision.T, np.ones((1, B), f)], axis=0)  # (4, B)
    txtT = np.concatenate([text.T, np.ones((1, B), f)], axis=0)  # (2, B)

    # knot = tanh(2*delta*(W x + b)) -> fold 2*delta into [W | b]
    dv = 2.0 * np.asarray(v_delta, dtype=f).reshape(DIM, 1)
    dt_ = 2.0 * np.asarray(t_delta, dtype=f).reshape(DIM, 1)
    vW_aug = dv * np.concatenate(
        [np.asarray(v_proj_w, f), np.asarray(v_proj_b, f).reshape(DIM, 1)], axis=1
    )  # (256, 4)
    tW_aug = dt_ * np.concatenate(
        [np.asarray(t_proj_w, f), np.asarray(t_proj_b, f).reshape(DIM, 1)], axis=1
    )  # (256, 2)

    FT = np.asarray(F_w, f).T  # (128,128) lhsT
    GT = np.asarray(G_w, f).T
    Fb = np.asarray(F_b, f).reshape(H)
    Gb = np.asarray(G_b, f).reshape(H)
    zero_bias = not (np.any(Fb) or np.any(Gb))

    # Q: u = Q @ t  with u[2i] = -t[2i+1], u[2i+1] = t[2i]; lhsT = Q.T
    Q = np.zeros((H, H), f)
    idx = np.arange(0, H, 2)
    Q[idx, idx + 1] = -1.0
    Q[idx + 1, idx] = 1.0

    wb = np.zeros((H, WBLOB_W), f)
    wb[:, _C_FT : _C_FT + H] = FT
    wb[:, _C_GT : _C_GT + H] = GT
    wb[:, _C_QT : _C_QT + H] = Q.T
    wb[:, _C_B + 0] = Fb * f(INV_SQRT2)
    wb[:, _C_B + 1] = Gb * f(INV_SQRT2)
    wb[:, _C_B + 2] = Fb
    wb[:, _C_B + 3] = Gb

    xbase = np.zeros((XBLOB_P, XBLOB_W), f)
    xbase[0:4, _C_W : _C_W + DIM] = vW_aug.T
    xbase[4:6, _C_W : _C_W + DIM] = tW_aug.T

    in_maps = []
    for c in range(NCORES):
        sl = slice(c * RPC, (c + 1) * RPC)
        xb = xbase.copy()
        xb[0:4, _C_X : _C_X + RPC] = visT[:, sl]
        xb[4:6, _C_X : _C_X + RPC] = txtT[:, sl]
        in_maps.append({"wblob": wb, "xblob": xb})
    return in_maps, zero_bias


def kernel(tmpdir=None, **inputs):
    global last_results
    in_maps, zero_bias = _prep_in_maps(**inputs)
    key = ("v2", zero_bias)
    if key not in _cached:
        _cached[key] = _build_nc(zero_bias)
    res = run_bass_kernel_spmd(_cached[key], in_maps, list(range(NCORES)), tmpdir=tmpdir)
    last_results = res
    # berry2d[p, c] holds row c*128+p of this core's 512-row block
    berry = np.concatenate(
        [np.ascontiguousarray(res.results[c]["berry2d"].T).reshape(RPC) for c in range(NCORES)]
    )
    eta_sum = np.sum([res.results[c]["eta_part"][0] for c in range(NCORES)], dtype=np.float64)
    eta = np.float32(eta_sum / (B * np.pi))
    return berry.astype(np.float32), eta
